# revision 4
# baseline (speedup 1.0000x reference)
"""LlamaAttention (B=2, S=2048, H=4096, 32 q heads / 8 kv heads, RoPE, causal)
on 8 Trainium2 NeuronCores.

Sharding: data-parallel over batch (2) x tensor-parallel over heads (4).
Core c = b*4 + t handles batch b with q heads 8t..8t+7 and kv heads 2t..2t+1.
Each core computes a partial output y_c = attn_out_local @ wo_local^T
([S, H], fp32); the host sums the 4 TP partials per batch.

Per-core kernel (all matmuls bf16 inputs, fp32 PSUM accumulation):
  phase Q : qT[d, s] per head  = (wqT chunk).T @ hsT chunk     (+ fused RoPE)
  phase KV: kT[d, s] per kv head (+ RoPE); vNat[s, d] (natural layout) with a
            ones column appended -> vAug, so the softmax denominator falls out
            of the AV matmul for free.
  phase A : scores computed transposed (sT[k, q] = kT_chunk.T @ qT) so that
            exp(sT) blocks feed the AV matmul directly as stationary operands.
            No max-subtraction (scores are O(1) for this data). Causal masking
            via 4 precomputed additive mask tiles on the diagonal band; fully
            masked blocks are skipped entirely.
  phase O : y[s, :] accumulated over the 8 local head-dim chunks.
"""
import sys

sys.path.insert(0, "/opt/trn_rl_repo")

import numpy as np
import ml_dtypes

BF16 = ml_dtypes.bfloat16

B, S, H = 2, 2048, 4096
NH, NKV, HD = 32, 8, 128
THETA = 10000.0
SCALE = 1.0 / float(np.sqrt(HD))

N_CORES = 8
TP = 4
NH_L = NH // TP      # 8 local q heads
NKV_L = NKV // TP    # 2 local kv heads
GRP_L = NH_L // NKV_L  # 4 q heads per local kv head
TOKB = 512
NKC = H // 128       # 32 contraction chunks
NTB = S // TOKB      # 4 token blocks
NQC = S // 128       # 16 token chunks
MASK_VAL = -10000.0
VSTRIDE = 132        # per-chunk stride in vAug (129 used, padded for alignment)

_NC_CACHE = {}


def _rope(nc, rp, psum, cos_sb, sinn_sb, tsl, outT, col0, f32, bf16):
    """RoPE on a [128(d), TOKB] fp32 PSUM block; writes bf16 into outT[:, col0+tsl].

    out[0:64]   = p[0:64]*cos - p[64:128]*sin
    out[64:128] = p[64:128]*cos + p[0:64]*sin
    (cos rows duplicated; sinn rows 0:64 pre-negated on host.)
    """
    tcos = rp.tile([128, TOKB], f32, tag="tcos")
    nc.vector.tensor_mul(tcos[:], psum[:], cos_sb[:, tsl])
    trs = rp.tile([128, TOKB], f32, tag="trs")
    nc.vector.tensor_mul(trs[0:64, :], psum[64:128, :], sinn_sb[0:64, tsl])
    nc.vector.tensor_mul(trs[64:128, :], psum[0:64, :], sinn_sb[64:128, tsl])
    nc.vector.tensor_add(outT[:, col0 + tsl.start : col0 + tsl.stop], tcos[:], trs[:])


def _build(reps=1):
    import concourse.mybir as mybir
    import concourse.tile as tile
    from concourse import bacc
    from contextlib import ExitStack

    dt = mybir.dt
    f32, bf16 = dt.float32, dt.bfloat16
    af = mybir.ActivationFunctionType

    nc = bacc.Bacc("TRN2", target_bir_lowering=False, debug=False,
                   enable_asserts=True, num_devices=N_CORES)
    hsT = nc.dram_tensor("hsT", [H, S], bf16, kind="ExternalInput").ap()
    wqT = nc.dram_tensor("wqT", [H, NH_L * HD], bf16, kind="ExternalInput").ap()
    wkT = nc.dram_tensor("wkT", [H, NKV_L * HD], bf16, kind="ExternalInput").ap()
    wvT = nc.dram_tensor("wvT", [H, NKV_L * HD], bf16, kind="ExternalInput").ap()
    woT = nc.dram_tensor("woT", [NH_L * HD, H], bf16, kind="ExternalInput").ap()
    cosT = nc.dram_tensor("cosT", [128, S], f32, kind="ExternalInput").ap()
    sinN = nc.dram_tensor("sinN", [128, S], f32, kind="ExternalInput").ap()
    maskT = nc.dram_tensor("maskT", [128, 4 * TOKB], f32, kind="ExternalInput").ap()
    ident = nc.dram_tensor("ident", [128, 128], bf16, kind="ExternalInput").ap()
    y = nc.dram_tensor("y", [S, H], f32, kind="ExternalOutput").ap()

    def emit(ctx, tc):
        ps = ctx.enter_context(tc.tile_pool(name="ps", bufs=8, space="PSUM"))
        persist = ctx.enter_context(tc.tile_pool(name="persist", bufs=1))

        mask_sb = persist.tile([128, 4 * TOKB], f32, tag="mask")
        nc.sync.dma_start(mask_sb[:], maskT[:])
        id_sb = persist.tile([128, 128], bf16, tag="ident")
        nc.sync.dma_start(id_sb[:], ident[:])
        qT = persist.tile([128, NH_L * S], bf16, tag="qT")
        kT = persist.tile([128, NKV_L * S], bf16, tag="kT")
        vA = persist.tile([128, NKV_L * NQC * VSTRIDE], bf16, tag="vA")
        nc.gpsimd.memset(vA[:], 1.0)

        # ---- phases Q and KV (stream hsT twice) ----
        with tc.tile_pool(name="cs", bufs=1) as csp, \
             tc.tile_pool(name="hs", bufs=8) as hsp, \
             tc.tile_pool(name="rope", bufs=4) as rp:
            cos_sb = csp.tile([128, S], f32, tag="cos")
            nc.sync.dma_start(cos_sb[:], cosT[:])
            sinn_sb = csp.tile([128, S], f32, tag="sinn")
            nc.sync.dma_start(sinn_sb[:], sinN[:])

            with tc.tile_pool(name="wq", bufs=1) as wqp:
                wq_sb = wqp.tile([128, NKC * NH_L * 128], bf16, tag="wq")
                for kc in range(NKC):
                    nc.sync.dma_start(
                        wq_sb[:, kc * 1024:(kc + 1) * 1024],
                        wqT[kc * 128:(kc + 1) * 128, :])
                for tb in range(NTB):
                    tsl = slice(tb * TOKB, (tb + 1) * TOKB)
                    pqs = [ps.tile([128, TOKB], f32, tag="ps", name=f"pq{tb}_{i}") for i in range(NH_L)]
                    for kc in range(NKC):
                        ht = hsp.tile([128, TOKB], bf16)
                        nc.sync.dma_start(ht[:], hsT[kc * 128:(kc + 1) * 128, tsl])
                        for h in range(NH_L):
                            c0 = kc * 1024 + h * 128
                            nc.tensor.matmul(pqs[h][:], wq_sb[:, c0:c0 + 128], ht[:],
                                             start=(kc == 0), stop=(kc == NKC - 1))
                    for h in range(NH_L):
                        _rope(nc, rp, pqs[h], cos_sb, sinn_sb, tsl, qT, h * S, f32, bf16)

            with tc.tile_pool(name="wkv", bufs=1) as wkvp:
                wk_sb = wkvp.tile([128, NKC * NKV_L * 128], bf16, tag="wk")
                wv_sb = wkvp.tile([128, NKC * NKV_L * 128], bf16, tag="wv")
                for kc in range(NKC):
                    nc.sync.dma_start(wk_sb[:, kc * 256:(kc + 1) * 256],
                                      wkT[kc * 128:(kc + 1) * 128, :])
                    nc.sync.dma_start(wv_sb[:, kc * 256:(kc + 1) * 256],
                                      wvT[kc * 128:(kc + 1) * 128, :])
                for tb in range(NTB):
                    tsl = slice(tb * TOKB, (tb + 1) * TOKB)
                    pks = [ps.tile([128, TOKB], f32, tag="ps", name=f"pk{tb}_{i}") for i in range(NKV_L)]
                    pvs = [ps.tile([128, NKV_L * 128], f32, tag="ps", name=f"pv{tb}_{i}") for i in range(4)]
                    for kc in range(NKC):
                        ht = hsp.tile([128, TOKB], bf16)
                        nc.sync.dma_start(ht[:], hsT[kc * 128:(kc + 1) * 128, tsl])
                        for g in range(NKV_L):
                            c0 = kc * 256 + g * 128
                            nc.tensor.matmul(pks[g][:], wk_sb[:, c0:c0 + 128], ht[:],
                                             start=(kc == 0), stop=(kc == NKC - 1))
                        for s in range(4):
                            nc.tensor.matmul(pvs[s][:], ht[:, s * 128:(s + 1) * 128],
                                             wv_sb[:, kc * 256:(kc + 1) * 256],
                                             start=(kc == 0), stop=(kc == NKC - 1))
                    for g in range(NKV_L):
                        _rope(nc, rp, pks[g], cos_sb, sinn_sb, tsl, kT, g * S, f32, bf16)
                    for s in range(4):
                        qc = tb * 4 + s
                        for g in range(NKV_L):
                            c0 = (g * NQC + qc) * VSTRIDE
                            nc.vector.tensor_copy(vA[:, c0:c0 + 128],
                                                  pvs[s][:, g * 128:(g + 1) * 128])

        # ---- phases A and O ----
        with tc.tile_pool(name="sc2", bufs=1) as sc2, \
             tc.tile_pool(name="exp", bufs=20) as ep, \
             tc.tile_pool(name="on", bufs=6) as onp, \
             tc.tile_pool(name="yout", bufs=6) as yp:
            wo_sb = sc2.tile([128, NH_L * H], bf16, tag="wo")
            for dc in range(NH_L):
                nc.sync.dma_start(wo_sb[:, dc * H:(dc + 1) * H],
                                  woT[dc * 128:(dc + 1) * 128, :])
            oT = sc2.tile([128, NH_L * S], bf16, tag="oT")

            for h in range(NH_L):
                g = h // GRP_L
                for qb in range(NTB):
                    nkc = 4 * qb + 4
                    exps = []
                    for kc in range(nkc):
                        ps_s = ps.tile([128, TOKB], f32, tag="ps")
                        nc.tensor.matmul(
                            ps_s[:],
                            kT[:, g * S + kc * 128: g * S + (kc + 1) * 128],
                            qT[:, h * S + qb * TOKB: h * S + (qb + 1) * TOKB],
                            start=True, stop=True)
                        r = kc - 4 * qb
                        if r >= 0:
                            nc.vector.tensor_add(ps_s[:], ps_s[:],
                                                 mask_sb[:, r * TOKB:(r + 1) * TOKB])
                        e = ep.tile([128, TOKB], bf16)
                        nc.scalar.activation(e[:], ps_s[:], af.Exp, scale=SCALE)
                        exps.append(e)
                    for s2 in range(4):
                        qc = 4 * qb + s2
                        po = ps.tile([128, VSTRIDE], f32, tag="ps")
                        for kc in range(qc + 1):
                            c0 = (g * NQC + kc) * VSTRIDE
                            nc.tensor.matmul(po[:, 0:129],
                                             exps[kc][:, s2 * 128:(s2 + 1) * 128],
                                             vA[:, c0:c0 + 129],
                                             start=(kc == 0), stop=(kc == qc))
                        rcp = onp.tile([128, 1], f32, tag="rcp")
                        nc.vector.reciprocal(rcp[:], po[:, 128:129])
                        on = onp.tile([128, 128], bf16, tag="on")
                        nc.vector.tensor_scalar_mul(on[:], po[:, 0:128], rcp[:])
                        pt = ps.tile([128, 128], bf16, tag="ps")
                        nc.tensor.transpose(pt[:], on[:], id_sb[:])
                        nc.vector.tensor_copy(
                            oT[:, h * S + qc * 128: h * S + (qc + 1) * 128], pt[:])

            for t in range(NQC):
                for hb in range(H // 512):
                    py = ps.tile([128, 512], f32, tag="ps")
                    for dc in range(NH_L):
                        nc.tensor.matmul(
                            py[:],
                            oT[:, dc * S + t * 128: dc * S + (t + 1) * 128],
                            wo_sb[:, dc * H + hb * 512: dc * H + (hb + 1) * 512],
                            start=(dc == 0), stop=(dc == NH_L - 1))
                    yt = yp.tile([128, 512], f32)
                    nc.vector.tensor_copy(yt[:], py[:])
                    nc.sync.dma_start(y[t * 128:(t + 1) * 128,
                                        hb * 512:(hb + 1) * 512], yt[:])

    with tile.TileContext(nc) as tc:
        if reps == 1:
            with ExitStack() as ctx:
                emit(ctx, tc)
        else:
            with tc.For_i(0, reps, 1):
                with ExitStack() as ctx:
                    emit(ctx, tc)
    nc.compile()
    return nc


def get_nc(reps=1):
    if reps not in _NC_CACHE:
        _NC_CACHE[reps] = _build(reps)
    return _NC_CACHE[reps]


def make_in_maps(hidden_states, position_ids, wq, wk, wv, wo):
    hidden_states = np.asarray(hidden_states, dtype=np.float32)
    position_ids = np.asarray(position_ids)
    wq = np.asarray(wq, dtype=np.float32)
    wk = np.asarray(wk, dtype=np.float32)
    wv = np.asarray(wv, dtype=np.float32)
    wo = np.asarray(wo, dtype=np.float32)

    # rope tables per batch
    j = np.arange(64, dtype=np.float64)
    invf = 1.0 / (THETA ** (2.0 * j / HD))       # [64]
    # mask tiles
    kp = np.arange(128)[:, None]
    qf = np.arange(TOKB)[None, :]
    maskT = np.empty((128, 4 * TOKB), dtype=np.float32)
    for r in range(4):
        maskT[:, r * TOKB:(r + 1) * TOKB] = np.where(
            qf >= kp + 128 * r, 0.0, MASK_VAL)
    ident = np.eye(128, dtype=BF16)

    in_maps = []
    for c in range(N_CORES):
        b, t = divmod(c, TP)
        pos = position_ids[b].astype(np.float64)     # [S]
        freqs = pos[:, None] * invf[None, :]         # [S, 64]
        cos64 = np.cos(freqs).astype(np.float32).T   # [64, S]
        sin64 = np.sin(freqs).astype(np.float32).T
        cosT = np.concatenate([cos64, cos64], axis=0)          # [128, S]
        sinN = np.concatenate([-sin64, sin64], axis=0)         # [128, S]
        in_maps.append({
            "hsT": np.ascontiguousarray(hidden_states[b].T).astype(BF16),
            "wqT": np.ascontiguousarray(
                wq[t * NH_L * HD:(t + 1) * NH_L * HD, :].T).astype(BF16),
            "wkT": np.ascontiguousarray(
                wk[t * NKV_L * HD:(t + 1) * NKV_L * HD, :].T).astype(BF16),
            "wvT": np.ascontiguousarray(
                wv[t * NKV_L * HD:(t + 1) * NKV_L * HD, :].T).astype(BF16),
            "woT": np.ascontiguousarray(
                wo[:, t * NH_L * HD:(t + 1) * NH_L * HD].T).astype(BF16),
            "cosT": np.ascontiguousarray(cosT),
            "sinN": np.ascontiguousarray(sinN),
            "maskT": maskT,
            "ident": ident,
        })
    return in_maps


def gather_out(results):
    """results: list of 8 dicts with 'y' [S, H] fp32 -> [B, S, H] fp32."""
    out = np.zeros((B, S, H), dtype=np.float32)
    for c in range(N_CORES):
        b = c // TP
        out[b] += results[c]["y"]
    return out


def kernel(**inputs):
    from concourse.bass_utils import run_bass_kernel_spmd

    nc = get_nc(reps=1)
    in_maps = make_in_maps(**inputs)
    res = run_bass_kernel_spmd(nc, in_maps, core_ids=list(range(N_CORES)))
    return gather_out(res.results)
